# revision 30
# baseline (speedup 1.0000x reference)
"""DglGraphConvolution Trainium2 kernel — dense block-adjacency SpMM.

Key idea: segment_sum over edges == A @ x where A[d, s] = multiplicity of
edge (s -> d). Host re-encodes the edge index lists as the dense count
matrix A^T (src-major, fp8 e4m3: counts are tiny ints, exactly
representable) — pure index preprocessing, no model data touched.

Because aggregation and the feature transform are both linear, the device
aggregates RAW text first and applies W after:

  per graph g (2 per core, data-parallel over B=16 on 8 cores):
    stage 1: for ws in 0..31:   (src windows of 128 nodes)
       tagg[fin, d] += text_ws[s, fin]-as-stationary @ A^T[s, d]
       (8 matmuls of 512 moving cols -> 8 psum banks; A^T row streamed
        straight from HBM as fp8 512 KiB rows on the SP DGE queue; text
        converted f32->bf16 on DVE; the stationary operand is reused
        across 8 matmuls so LDWEIGHTS pipelines away)
    evict: tagg_n = tagg * recip_deg[d]   (DVE, psum -> bf16 sbuf)
    stage 2: out^T[f, d] = W-as-stationary @ tagg_n[fin, d]
    evict: out = out^T + bias[f]          (ACT per-partition bias, bf16)

deg comes from the same index-only host prep (recip = 1/(deg+1), bf16,
replicated across the 128 partitions). Output is written transposed
[f, d] and untransposed on the host.
"""

import numpy as np

B, N, E, F = 16, 4096, 131072, 128
NCORES = 8
GPC = B // NCORES  # graphs per core
W = 128  # src window (partition) size
NW = N // W  # 32 src windows
Q = 8  # moving-dim quadrants per A row
QW = N // Q  # 512 moving columns per matmul (= MAX_MOVING_FREE_DIM_SIZE)

_cache = {}


def _build_program():
    from contextlib import ExitStack

    import concourse.bacc as bacc
    import concourse.tile as tile
    from concourse import mybir
    from concourse._compat import get_trn_type

    f32 = mybir.dt.float32
    bf16 = mybir.dt.bfloat16
    fp8 = mybir.dt.float8e4

    nc = bacc.Bacc(get_trn_type() or "TRN2", target_bir_lowering=False, debug=False)

    text_d = nc.dram_tensor("text", [GPC, N, F], f32, kind="ExternalInput")
    a_d = nc.dram_tensor("acnt", [GPC, NW, W, N], fp8, kind="ExternalInput")
    rec_d = nc.dram_tensor("recrep", [GPC, W, N], bf16, kind="ExternalInput")
    w_d = nc.dram_tensor("weight", [F, F], f32, kind="ExternalInput")
    bias_d = nc.dram_tensor("biascol", [F, 1], f32, kind="ExternalInput")
    out_d = nc.dram_tensor("out", [GPC, F, N], bf16, kind="ExternalOutput")

    with tile.TileContext(nc) as tc, ExitStack() as ctx:
        const = ctx.enter_context(tc.tile_pool(name="const", bufs=1))
        tpool = ctx.enter_context(tc.tile_pool(name="tpool", bufs=4))
        spool = ctx.enter_context(tc.tile_pool(name="spool", bufs=2))
        apool = ctx.enter_context(tc.tile_pool(name="apool", bufs=14))
        gpool = ctx.enter_context(tc.tile_pool(name="gpool", bufs=2))
        opool = ctx.enter_context(tc.tile_pool(name="opool", bufs=3))
        psum = ctx.enter_context(tc.tile_pool(name="psum", bufs=8, space="PSUM"))

        # const DMAs are deferred into the loop so they do not sit ahead of
        # the first A rows in the cold DGE queue
        w_sb = const.tile([F, F], f32)
        w_bf = const.tile([F, F], bf16)
        bias_sb = const.tile([F, 1], f32)

        # warm-up: dummy matmuls while the first A rows are still in
        # flight, so the PE p-state ramp (0.65 -> 2.4 GHz after 3 us of
        # continuous execution) is paid before the real stream starts
        dum_w = const.tile([W, F], bf16)
        nc.vector.memset(dum_w[:], 0.0)
        dum_m = const.tile([W, QW], bf16)
        nc.vector.memset(dum_m[:], 0.0)
        warm_ps = psum.tile([W, QW], f32, tag="acc", name="warm")
        for _ in range(28):
            nc.tensor.matmul(
                out=warm_ps[:], lhsT=dum_w[:], rhs=dum_m[:], start=True, stop=True
            )

        for g in range(GPC):
            recrep = gpool.tile([W, N], bf16, tag="rec")
            st_all = spool.tile([W, NW * F], bf16, tag="s", name=f"st{g}")

            acc = []
            for ws in range(NW):
                tt = tpool.tile([W, F], f32, tag="t")
                # text rides the ACT DGE queue so the SP queue is a pure
                # A-row stream (halves the cold-start supply cadence)
                nc.scalar.dma_start(tt[:], text_d[g, W * ws : W * (ws + 1), :])
                st = st_all[:, F * ws : F * (ws + 1)]
                nc.vector.tensor_copy(st, tt[:])
                if g == 0 and ws == 0:
                    # split the very first row so the first matmuls only
                    # wait on 256 KiB
                    ar = apool.tile([W, N], fp8, tag="a", name="a00")
                    nc.sync.dma_start(ar[:, 0 : N // 2], a_d[g, ws, :, 0 : N // 2])
                    nc.sync.dma_start(ar[:, N // 2 : N], a_d[g, ws, :, N // 2 : N])
                else:
                    ar = apool.tile([W, N], fp8, tag="a", name=f"a{g}_{ws}")
                    nc.sync.dma_start(ar[:], a_d[g, ws])
                if g == 0 and ws == 16:
                    nc.scalar.dma_start(w_sb[:], w_d[:, :])
                    nc.vector.tensor_copy(w_bf[:], w_sb[:])
                    nc.scalar.dma_start(bias_sb[:], bias_d[:, :])
                if ws == 8:
                    # deferred: not needed until psum eviction, keep the
                    # head of the stream free for the first A rows
                    nc.scalar.dma_start(recrep[:], rec_d[g])
                for q in range(Q):
                    if ws == 0:
                        a_ps = psum.tile([W, QW], f32, tag="acc", name=f"acc{g}_{q}")
                        acc.append(a_ps)
                    nc.tensor.matmul(
                        out=acc[q][:],
                        lhsT=st,
                        rhs=ar[:, QW * q : QW * (q + 1)],
                        start=(ws == 0),
                        stop=(ws == NW - 1),
                    )

            tagg = gpool.tile([F, N], bf16, tag="tagg")
            for q in range(Q):
                nc.vector.tensor_tensor(
                    out=tagg[:, QW * q : QW * (q + 1)],
                    in0=acc[q][:],
                    in1=recrep[:, QW * q : QW * (q + 1)],
                    op=mybir.AluOpType.mult,
                )

            for q in range(Q):
                o_ps = psum.tile([F, QW], f32, tag="acc", name=f"ops{g}_{q}")
                nc.tensor.matmul(
                    out=o_ps[:],
                    lhsT=w_bf[:],
                    rhs=tagg[:, QW * q : QW * (q + 1)],
                    start=True,
                    stop=True,
                )
                obf = opool.tile([F, QW], bf16, tag="o")
                nc.scalar.activation(
                    obf[:],
                    o_ps[:],
                    mybir.ActivationFunctionType.Identity,
                    bias=bias_sb[:, 0:1],
                )
                nc.sync.dma_start(out_d[g, :, QW * q : QW * (q + 1)], obf[:])

    nc.compile()
    return nc


def _prep_graph(src, dst):
    """Index-only: dense src-major count matrix [NW, W, N] (float32 counts)
    and the replicated reciprocal degree row [W, N]."""
    lin = src.astype(np.int64) * N + dst
    cnt = np.bincount(lin, minlength=N * N).astype(np.float32)
    assert cnt.max() <= 16, f"edge multiplicity overflow: {cnt.max()}"
    deg = np.bincount(dst, minlength=N).astype(np.float32)
    rec = (1.0 / (deg + 1.0)).astype(np.float32)
    recrep = np.ascontiguousarray(np.broadcast_to(rec[None, :], (W, N)))
    return cnt.reshape(NW, W, N), recrep


def kernel(text, weight, bias, edge_src, edge_dst):
    import ml_dtypes

    text = np.asarray(text, dtype=np.float32)
    weight = np.asarray(weight, dtype=np.float32)
    bias = np.asarray(bias, dtype=np.float32)
    edge_src = np.asarray(edge_src, dtype=np.int32)
    edge_dst = np.asarray(edge_dst, dtype=np.int32)

    if "nc" not in _cache:
        _cache["nc"] = _build_program()
    nc = _cache["nc"]

    in_maps = []
    for k in range(NCORES):
        acnt = np.empty((GPC, NW, W, N), dtype=ml_dtypes.float8_e4m3)
        recrep = np.empty((GPC, W, N), dtype=ml_dtypes.bfloat16)
        for g in range(GPC):
            b = k * GPC + g
            cnt, rr = _prep_graph(edge_src[b], edge_dst[b])
            acnt[g] = cnt.astype(ml_dtypes.float8_e4m3)
            recrep[g] = rr
        in_maps.append(
            {
                "text": text[k * GPC : (k + 1) * GPC],
                "acnt": acnt,
                "recrep": recrep,
                "weight": weight,
                "biascol": bias.reshape(F, 1),
            }
        )

    _cache["in_maps"] = in_maps

    from concourse.bass_utils import run_bass_kernel_spmd

    res = run_bass_kernel_spmd(nc, in_maps, list(range(NCORES)))
    out = np.concatenate(
        [
            np.asarray(res.results[k]["out"])
            .astype(np.float32)
            .transpose(0, 2, 1)
            for k in range(NCORES)
        ],
        axis=0,
    )
    return out


# revision 33
# speedup vs baseline: 1.2644x; 1.2644x over previous
"""DglGraphConvolution Trainium2 kernel — dense block-adjacency SpMM.

Key idea: segment_sum over edges == A @ x where A[d, s] = multiplicity of
edge (s -> d). Host re-encodes the edge index lists as the dense count
matrix A^T (src-major, fp8 e4m3: counts are tiny ints, exactly
representable) — pure index preprocessing, no model data touched.

Because aggregation and the feature transform are both linear, the device
aggregates RAW text first and applies W after:

  per graph g (2 per core, data-parallel over B=16 on 8 cores):
    stage 1: for ws in 0..31:   (src windows of 128 nodes)
       tagg[fin, d] += text_ws[s, fin]-as-stationary @ A^T[s, d]
       (8 matmuls of 512 moving cols -> 8 psum banks; A^T row streamed
        straight from HBM as fp8 512 KiB rows on the SP DGE queue; text
        converted f32->bf16 on DVE; the stationary operand is reused
        across 8 matmuls so LDWEIGHTS pipelines away)
    evict: tagg_n = tagg * recip_deg[d]   (DVE, psum -> bf16 sbuf)
    stage 2: out^T[f, d] = W-as-stationary @ tagg_n[fin, d]
    evict: out = out^T + bias[f]          (ACT per-partition bias, bf16)

deg comes from the same index-only host prep (recip = 1/(deg+1), bf16,
replicated across the 128 partitions). Output is written transposed
[f, d] and untransposed on the host.
"""

import numpy as np

B, N, E, F = 16, 4096, 131072, 128
NCORES = 8
GPC = B // NCORES  # graphs per core
W = 128  # src window (partition) size
NW = N // W  # 32 src windows
Q = 8  # moving-dim quadrants per A row
QW = N // Q  # 512 moving columns per matmul (= MAX_MOVING_FREE_DIM_SIZE)

_cache = {}


def _build_program():
    from contextlib import ExitStack

    import concourse.bacc as bacc
    import concourse.tile as tile
    from concourse import mybir
    from concourse._compat import get_trn_type

    f32 = mybir.dt.float32
    bf16 = mybir.dt.bfloat16
    fp8 = mybir.dt.float8e4

    nc = bacc.Bacc(get_trn_type() or "TRN2", target_bir_lowering=False, debug=False)

    text_d = nc.dram_tensor("text", [GPC, N, F], f32, kind="ExternalInput")
    a_d = nc.dram_tensor("acnt", [GPC, NW, W, N], fp8, kind="ExternalInput")
    rec_d = nc.dram_tensor("recrep", [GPC, W, N], bf16, kind="ExternalInput")
    w_d = nc.dram_tensor("weight", [F, F], f32, kind="ExternalInput")
    bias_d = nc.dram_tensor("biascol", [F, 1], f32, kind="ExternalInput")
    out_d = nc.dram_tensor("out", [GPC, F, N], bf16, kind="ExternalOutput")

    with tile.TileContext(nc) as tc, ExitStack() as ctx:
        const = ctx.enter_context(tc.tile_pool(name="const", bufs=1))
        tpool = ctx.enter_context(tc.tile_pool(name="tpool", bufs=4))
        spool = ctx.enter_context(tc.tile_pool(name="spool", bufs=3))
        apool = ctx.enter_context(tc.tile_pool(name="apool", bufs=14))
        gpool = ctx.enter_context(tc.tile_pool(name="gpool", bufs=2))
        opool = ctx.enter_context(tc.tile_pool(name="opool", bufs=3))
        psum = ctx.enter_context(tc.tile_pool(name="psum", bufs=8, space="PSUM"))

        # const DMAs are deferred into the loop so they do not sit ahead of
        # the first A rows in the cold DGE queue
        w_sb = const.tile([F, F], f32)
        w_bf = const.tile([F, F], bf16)
        bias_sb = const.tile([F, 1], f32)

        # warm-up: dummy matmuls while the first A rows are still in
        # flight, so the PE p-state ramp (0.65 -> 2.4 GHz after 3 us of
        # continuous execution) is paid before the real stream starts
        dum_w = const.tile([W, F], bf16)
        nc.vector.memset(dum_w[:], 0.0)
        dum_m = const.tile([W, QW], bf16)
        nc.vector.memset(dum_m[:], 0.0)
        warm_ps = psum.tile([W, QW], f32, tag="acc", name="warm")
        for _ in range(28):
            nc.tensor.matmul(
                out=warm_ps[:], lhsT=dum_w[:], rhs=dum_m[:], start=True, stop=True
            )

        for g in range(GPC):
            recrep = gpool.tile([W, N], bf16, tag="rec")

            acc = []
            for ws in range(NW):
                tt = tpool.tile([W, F], f32, tag="t")
                # text rides the ACT DGE queue so the SP queue is a pure
                # A-row stream (halves the cold-start supply cadence)
                nc.scalar.dma_start(tt[:], text_d[g, W * ws : W * (ws + 1), :])
                st = spool.tile([W, F], bf16, tag="s")
                nc.vector.tensor_copy(st[:], tt[:])
                if g == 0 and ws == 0:
                    # split the very first row so the first matmuls only
                    # wait on 256 KiB
                    ar = apool.tile([W, N], fp8, tag="a", name="a00")
                    nc.sync.dma_start(ar[:, 0 : N // 2], a_d[g, ws, :, 0 : N // 2])
                    nc.sync.dma_start(ar[:, N // 2 : N], a_d[g, ws, :, N // 2 : N])
                else:
                    ar = apool.tile([W, N], fp8, tag="a", name=f"a{g}_{ws}")
                    nc.sync.dma_start(ar[:], a_d[g, ws])
                if g == 0 and ws == 16:
                    nc.scalar.dma_start(w_sb[:], w_d[:, :])
                    nc.vector.tensor_copy(w_bf[:], w_sb[:])
                    nc.scalar.dma_start(bias_sb[:], bias_d[:, :])
                if ws == 8:
                    # deferred: not needed until psum eviction, keep the
                    # head of the stream free for the first A rows
                    nc.scalar.dma_start(recrep[:], rec_d[g])
                for q in range(Q):
                    if ws == 0:
                        a_ps = psum.tile([W, QW], f32, tag="acc", name=f"acc{g}_{q}")
                        acc.append(a_ps)
                    nc.tensor.matmul(
                        out=acc[q][:],
                        lhsT=st[:],
                        rhs=ar[:, QW * q : QW * (q + 1)],
                        start=(ws == 0),
                        stop=(ws == NW - 1),
                    )

            tagg = gpool.tile([F, N], bf16, tag="tagg")
            for q in range(Q):
                nc.vector.tensor_tensor(
                    out=tagg[:, QW * q : QW * (q + 1)],
                    in0=acc[q][:],
                    in1=recrep[:, QW * q : QW * (q + 1)],
                    op=mybir.AluOpType.mult,
                )

            for q in range(Q):
                o_ps = psum.tile([F, QW], f32, tag="acc", name=f"ops{g}_{q}")
                nc.tensor.matmul(
                    out=o_ps[:],
                    lhsT=w_bf[:],
                    rhs=tagg[:, QW * q : QW * (q + 1)],
                    start=True,
                    stop=True,
                )
                obf = opool.tile([F, QW], bf16, tag="o")
                nc.scalar.activation(
                    obf[:],
                    o_ps[:],
                    mybir.ActivationFunctionType.Identity,
                    bias=bias_sb[:, 0:1],
                )
                nc.sync.dma_start(out_d[g, :, QW * q : QW * (q + 1)], obf[:])

    nc.compile()
    return nc


def _prep_graph(src, dst):
    """Index-only: dense src-major count matrix [NW, W, N] (float32 counts)
    and the replicated reciprocal degree row [W, N]."""
    lin = src.astype(np.int64) * N + dst
    cnt = np.bincount(lin, minlength=N * N).astype(np.float32)
    assert cnt.max() <= 16, f"edge multiplicity overflow: {cnt.max()}"
    deg = np.bincount(dst, minlength=N).astype(np.float32)
    rec = (1.0 / (deg + 1.0)).astype(np.float32)
    recrep = np.ascontiguousarray(np.broadcast_to(rec[None, :], (W, N)))
    return cnt.reshape(NW, W, N), recrep


def kernel(text, weight, bias, edge_src, edge_dst):
    import ml_dtypes

    text = np.asarray(text, dtype=np.float32)
    weight = np.asarray(weight, dtype=np.float32)
    bias = np.asarray(bias, dtype=np.float32)
    edge_src = np.asarray(edge_src, dtype=np.int32)
    edge_dst = np.asarray(edge_dst, dtype=np.int32)

    if "nc" not in _cache:
        _cache["nc"] = _build_program()
    nc = _cache["nc"]

    in_maps = []
    for k in range(NCORES):
        acnt = np.empty((GPC, NW, W, N), dtype=ml_dtypes.float8_e4m3)
        recrep = np.empty((GPC, W, N), dtype=ml_dtypes.bfloat16)
        for g in range(GPC):
            b = k * GPC + g
            cnt, rr = _prep_graph(edge_src[b], edge_dst[b])
            acnt[g] = cnt.astype(ml_dtypes.float8_e4m3)
            recrep[g] = rr
        in_maps.append(
            {
                "text": text[k * GPC : (k + 1) * GPC],
                "acnt": acnt,
                "recrep": recrep,
                "weight": weight,
                "biascol": bias.reshape(F, 1),
            }
        )

    _cache["in_maps"] = in_maps

    from concourse.bass_utils import run_bass_kernel_spmd

    res = run_bass_kernel_spmd(nc, in_maps, list(range(NCORES)))
    out = np.concatenate(
        [
            np.asarray(res.results[k]["out"])
            .astype(np.float32)
            .transpose(0, 2, 1)
            for k in range(NCORES)
        ],
        axis=0,
    )
    return out
